# revision 44
# baseline (speedup 1.0000x reference)
"""Trainium2 Bass kernel for nn_Experts (topk_masking).

Math (reference):
  R = concat(h,us,ue) @ W_r.T + b_r                       [1,1,512]
  x = concat(u, R.broadcast)                              [1,S,1536]
  h1 = (x @ W_nn.T + b_nn).reshape(S,512,16)
  h2 = (x @ W_no.T + b_no).reshape(S,512,16) * noise
  g  = top2-masked softmax over experts of (h1+h2)
  e  = (x @ W_E.T + b_E).reshape(S,512,16)
  out = (g*e).mean(-1)                                    [1,S,512]

Sharding: the NE*DIM output-feature dim of the three projections is sharded
across 8 cores (64 dims x 16 experts each, contiguous feature slice). The
token-independent R-path is computed on the host in fp64 and folded into a
per-feature constant c[f]; the device contracts only over u's 1024 features.

Precision: gating logits accumulate in one PSUM chain at scale 2^18:
  fp16(x) @ fp16(W*2^18)                 main term (<=11-bit products, exact)
  e4m3(xl*2^14) @ e4m3(wA*2^4)           x-residual term, fp8 DoubleRow
  e4m3(x*2^3) @ e4m3(wl*2^15)            W-residual term, fp8 DoubleRow
The gating bias constants ride a host-precomputed (c_nn + c_no*noise)*2^18
tensor added during the m-assembly on the vector engine (no bias matmuls).
The 2^-18 descale folds into exp()'s scale operand; max/compare ops are
scale-invariant. This lands gating logits at ~2^-16 relative accuracy so
top-2 selection matches the fp32 reference.

The e-projection runs as three fp8 DoubleRow passes (xa@wa + xa@wb + xb@wa,
8-bit effective operands, dropping the 2^-8 cross term) sharing one PSUM
chain at scale 2^10; the descale folds into the output mean scalar.
"""
import numpy as np
import ml_dtypes

DIM = 512
NE = 16
S = 4096
KU = 2 * DIM        # u features = 1024
NCORES = 8
DL = DIM // NCORES  # 64 dims per core
FL = DL * NE        # 1024 features per core
MCH = S // 128      # 32 token chunks

SC = 2.0 ** 18      # gating PSUM scale
ISC = 2.0 ** -18

_MASK11 = np.uint32(0xFFFFF000)  # keep 11 explicit mantissa bits

TRACE = False
_CACHE = {}


def _trunc11(a):
    a = np.ascontiguousarray(a, dtype=np.float32)
    return (a.view(np.uint32) & _MASK11).view(np.float32)


def _build():
    import concourse.bass as bass
    import concourse.mybir as mybir
    import concourse.tile as tile
    from concourse import bacc
    from contextlib import ExitStack

    F32 = mybir.dt.float32
    F32R = mybir.dt.float32r
    F16 = mybir.dt.float16
    F8 = mybir.dt.float8e4
    AX = mybir.AxisListType
    OP = mybir.AluOpType
    ACTF = mybir.ActivationFunctionType
    DR = mybir.MatmulPerfMode.DoubleRow

    nc = bacc.Bacc("TRN2", target_bir_lowering=False, debug=False,
                   num_devices=NCORES)

    def dram(name, shape, dt, kind="ExternalInput"):
        return nc.dram_tensor(name, shape, dt, kind=kind)

    # per-core inputs (same names on every core; data differs per core)
    xhT = dram("xhT", [128, 8, S], F16)           # fp16(u), k = kc*128+p
    xl8T = dram("xl8T", [128, 2, 4, S], F8)       # e4m3(xl*2^14), k=256j+128i+p
    x88T = dram("x88T", [128, 2, 4, S], F8)       # e4m3(u*2^3)
    xb8T = dram("xb8T", [128, 2, 4, S], F8)       # e4m3((u - x88/2^3)*2^3)
    noise_c = dram("noise_c", [S, FL], F32)
    nzc_c = dram("nzc_c", [S, FL], F32)           # (c_nn + c_no*noise)*2^18
    whnn16 = dram("whnn16", [128, 8, FL], F16)    # fp16(W_nn.T*2^18)
    whno16 = dram("whno16", [128, 8, FL], F16)
    wea8 = dram("wea8", [128, 2, 4, FL], F8)      # e4m3(W_E.T*2^7)
    web8 = dram("web8", [128, 2, 4, FL], F8)      # e4m3((W_E.T-wea/2^7)*2^7)
    wh8nn = dram("wh8nn", [128, 2, 4, FL], F8)    # e4m3(wA_nn.T*2^4)
    wl8nn = dram("wl8nn", [128, 2, 4, FL], F8)    # e4m3(wl_nn.T*2^15)
    wh8no = dram("wh8no", [128, 2, 4, FL], F8)
    wl8no = dram("wl8no", [128, 2, 4, FL], F8)
    ccd = dram("ccd", [2, FL], F32R)              # trunc11 rows of c_E*2^10
    out_c = dram("out_c", [S, DL], F32, kind="ExternalOutput")

    with tile.TileContext(nc) as tc, ExitStack() as ctx:
        wpool = ctx.enter_context(tc.tile_pool(name="w", bufs=1))

        wea8_t = wpool.tile([128, 2, 4, FL], F8)
        web8_t = wpool.tile([128, 2, 4, FL], F8)
        whno_t = wpool.tile([128, 8, FL], F16)
        wh8no_t = wpool.tile([128, 2, 4, FL], F8)
        wl8no_t = wpool.tile([128, 2, 4, FL], F8)
        whnn_t = wpool.tile([128, 8, FL], F16)
        wh8nn_t = wpool.tile([128, 2, 4, FL], F8)
        wl8nn_t = wpool.tile([128, 2, 4, FL], F8)
        ccsb = wpool.tile([2, FL], F32R)

        onesf = wpool.tile([2, 128], F32)
        nc.vector.memset(onesf[:], 1.0)
        ones2 = wpool.tile([2, 128], F32R)
        nc.vector.tensor_copy(ones2[:], onesf[:])

        spool = ctx.enter_context(tc.tile_pool(name="stream", bufs=2))
        epool = ctx.enter_context(tc.tile_pool(name="epi", bufs=2))
        phpool = ctx.enter_context(tc.tile_pool(name="phps", bufs=1,
                                                space="PSUM"))
        pepool = ctx.enter_context(tc.tile_pool(name="peps", bufs=1,
                                                space="PSUM"))

        def fetch_x(m):
            tsl = slice(m * 128, (m + 1) * 128)
            xh_t = spool.tile([128, 8, 128], F16, tag="xh")
            xl8_t = spool.tile([128, 2, 4, 128], F8, tag="xl8")
            x88_t = spool.tile([128, 2, 4, 128], F8, tag="x88")
            xb8_t = spool.tile([128, 2, 4, 128], F8, tag="xb8")
            nz_t = spool.tile([128, FL], F32, tag="nz")
            nzc_t = spool.tile([128, FL], F32, tag="nzc")
            nc.sync.dma_start(xh_t[:], xhT.ap()[:, :, tsl])
            nc.sync.dma_start(xl8_t[:], xl8T.ap()[:, :, :, tsl])
            nc.sync.dma_start(x88_t[:], x88T.ap()[:, :, :, tsl])
            nc.sync.dma_start(xb8_t[:], xb8T.ap()[:, :, :, tsl])
            nc.sync.dma_start(nz_t[:], noise_c.ap()[tsl, :])
            nc.sync.dma_start(nzc_t[:], nzc_c.ap()[tsl, :])
            return xh_t, xl8_t, x88_t, xb8_t, nz_t, nzc_t

        # weight DMAs split per-k and interleaved with the chunk-0/1 input
        # prefetch, ordered to match the PE's consumption order (h2, h1, e)
        # so chunk 0 can start ~2us in instead of waiting for all weights.
        def dmaq(dst, src):
            nc.sync.dma_start(dst, src)

        def fetch_xin(m, q=False):
            tsl = slice(m * 128, (m + 1) * 128)
            xh_t = spool.tile([128, 8, 128], F16, tag="xh")
            xl8_t = spool.tile([128, 2, 4, 128], F8, tag="xl8")
            x88_t = spool.tile([128, 2, 4, 128], F8, tag="x88")
            xb8_t = spool.tile([128, 2, 4, 128], F8, tag="xb8")
            dma = dmaq if q else (lambda d, s: nc.sync.dma_start(d, s))
            dma(xh_t[:], xhT.ap()[:, :, tsl])
            dma(xl8_t[:], xl8T.ap()[:, :, :, tsl])
            dma(x88_t[:], x88T.ap()[:, :, :, tsl])
            dma(xb8_t[:], xb8T.ap()[:, :, :, tsl])
            return xh_t, xl8_t, x88_t, xb8_t

        def fetch_nz(m, q=False):
            tsl = slice(m * 128, (m + 1) * 128)
            nz_t = spool.tile([128, FL], F32, tag="nz")
            nzc_t = spool.tile([128, FL], F32, tag="nzc")
            dma = dmaq if q else (lambda d, s: nc.sync.dma_start(d, s))
            dma(nz_t[:], noise_c.ap()[tsl, :])
            dma(nzc_t[:], nzc_c.ap()[tsl, :])
            return nz_t, nzc_t

        # chunk 0: only xh and the first main-weight chunk gate the PE start;
        # the fp8 DR inputs are not needed until ~9 matmuls later
        tsl0 = slice(0, 128)
        xh_t0 = spool.tile([128, 8, 128], F16, tag="xh")
        dmaq(xh_t0[:], xhT.ap()[:, :, tsl0])
        for k in range(2):
            dmaq(whno_t[:, k, :], whno16.ap()[:, k, :])
        xl8_t0 = spool.tile([128, 2, 4, 128], F8, tag="xl8")
        x88_t0 = spool.tile([128, 2, 4, 128], F8, tag="x88")
        xb8_t0 = spool.tile([128, 2, 4, 128], F8, tag="xb8")
        dmaq(xl8_t0[:], xl8T.ap()[:, :, :, tsl0])
        dmaq(x88_t0[:], x88T.ap()[:, :, :, tsl0])
        dmaq(xb8_t0[:], xb8T.ap()[:, :, :, tsl0])
        x0 = (xh_t0, xl8_t0, x88_t0, xb8_t0)
        for k in range(2, 8):
            dmaq(whno_t[:, k, :], whno16.ap()[:, k, :])
        for j in range(4):
            dmaq(wh8no_t[:, :, j, :], wh8no.ap()[:, :, j, :])
            dmaq(wl8no_t[:, :, j, :], wl8no.ap()[:, :, j, :])
        dmaq(ccsb[:], ccd.ap())
        xq = [x0 + fetch_nz(0, q=True)]
        for k in range(8):
            dmaq(whnn_t[:, k, :], whnn16.ap()[:, k, :])
        for j in range(4):
            dmaq(wh8nn_t[:, :, j, :], wh8nn.ap()[:, :, j, :])
            dmaq(wl8nn_t[:, :, j, :], wl8nn.ap()[:, :, j, :])
        xq.append(fetch_xin(1, q=True) + fetch_nz(1, q=True))
        for j in range(4):
            dmaq(wea8_t[:, :, j, :], wea8.ap()[:, :, j, :])
            dmaq(web8_t[:, :, j, :], web8.ap()[:, :, j, :])

        def gating_phase(pt, xh_t, xl8_t, x88_t, w16_t, w8h_t, w8l_t):
            # bias constants ride the host-precomputed nzc tensor instead of
            # ones-matmuls, so the psum chain ends on the last DR residual
            for k in range(8):
                st = (k == 0)
                for half in range(2):
                    fsl = slice(half * 512, (half + 1) * 512)
                    nc.tensor.matmul(pt[:, fsl], xh_t[:, k, :],
                                     w16_t[:, k, fsl], start=st, stop=False)
            for j in range(4):
                for half in range(2):
                    fsl = slice(half * 512, (half + 1) * 512)
                    nc.tensor.matmul(pt[:, fsl], xl8_t[:, :, j, :],
                                     w8h_t[:, :, j, fsl],
                                     start=False, stop=False, perf_mode=DR)
                    nc.tensor.matmul(pt[:, fsl], x88_t[:, :, j, :],
                                     w8l_t[:, :, j, fsl],
                                     start=False, stop=(j == 3), perf_mode=DR)

        def gating_stage(m):
            """h2 + h1 phases, noise fold, and the top-2 mask chain."""
            xh_t, xl8_t, x88_t, xb8_t, nz_t, nzc_t = xq[m]
            if m + 2 < MCH:
                xq.append(fetch_x(m + 2))

            h1p = phpool.tile([128, FL], F32, tag="h1")
            h2p = phpool.tile([128, FL], F32, tag="h2")

            gating_phase(h2p, xh_t, xl8_t, x88_t, whno_t, wh8no_t, wl8no_t)
            t_t = epool.tile([128, FL], F32, tag="t")
            nc.vector.tensor_mul(t_t[:], h2p[:], nz_t[:])

            gating_phase(h1p, xh_t, xl8_t, x88_t, whnn_t, wh8nn_t, wl8nn_t)
            b_t = epool.tile([128, FL], F32, tag="B")
            nc.vector.tensor_add(b_t[:], h1p[:], nzc_t[:])
            m_t = epool.tile([128, FL], F32, tag="m")
            nc.vector.tensor_add(m_t[:], t_t[:], b_t[:])

            # top-2 mask chain runs on DVE while the PE moves on
            mg = m_t[:].rearrange("p (d e) -> p d e", e=NE)
            v1 = epool.tile([128, DL], F32, tag="v1")
            nc.vector.tensor_reduce(v1[:], mg, AX.X, op=OP.max)
            eq1 = epool.tile([128, FL], F32, tag="eq1")
            nc.vector.tensor_tensor(eq1[:].rearrange("p (d e) -> p d e", e=NE),
                                    mg, v1[:].broadcast_to([128, DL, NE]),
                                    OP.is_equal)
            m2 = epool.tile([128, FL], F32, tag="m2")
            nc.vector.scalar_tensor_tensor(m2[:], eq1[:], -1e30, m_t[:],
                                           OP.mult, OP.add)
            v2 = epool.tile([128, DL], F32, tag="v2")
            nc.vector.tensor_reduce(v2[:], m2[:].rearrange("p (d e) -> p d e",
                                                           e=NE),
                                    AX.X, op=OP.max)
            mask = epool.tile([128, FL], F32, tag="mask")
            nc.vector.tensor_tensor(mask[:].rearrange("p (d e) -> p d e", e=NE),
                                    mg, v2[:].broadcast_to([128, DL, NE]),
                                    OP.is_ge)
            q = epool.tile([128, FL], F32, tag="q")
            nc.scalar.activation(q[:], m_t[:], ACTF.Exp, scale=ISC)
            return (m, x88_t, xb8_t, v1, v2, mask, q)

        def finish_stage(c, half_major):
            """e phase (fp8 DoubleRow, psum scale 2^10) and the
            masked-softmax combine + output."""
            m, x88_t, xb8_t, v1, v2, mask, q = c
            tsl = slice(m * 128, (m + 1) * 128)
            s_t = epool.tile([128, DL], F32, tag="s")
            if not half_major:
                ep = pepool.tile([128, FL], F32, tag="e")
                for j in range(4):
                    st = (j == 0)
                    for half in range(2):
                        fsl = slice(half * 512, (half + 1) * 512)
                        nc.tensor.matmul(ep[:, fsl], x88_t[:, :, j, :],
                                         wea8_t[:, :, j, fsl],
                                         start=st, stop=False, perf_mode=DR)
                        nc.tensor.matmul(ep[:, fsl], x88_t[:, :, j, :],
                                         web8_t[:, :, j, fsl],
                                         start=False, stop=False,
                                         perf_mode=DR)
                        nc.tensor.matmul(ep[:, fsl], xb8_t[:, :, j, :],
                                         wea8_t[:, :, j, fsl],
                                         start=False, stop=False,
                                         perf_mode=DR)
                for half in range(2):
                    fsl = slice(half * 512, (half + 1) * 512)
                    nc.tensor.matmul(ep[:, fsl], ones2[:],
                                     ccsb[:, half * 512:(half + 1) * 512],
                                     start=False, stop=True)

                t1 = epool.tile([128, FL], F32, tag="t1")
                nc.vector.tensor_mul(t1[:], mask[:], ep[:])
                t2 = epool.tile([128, FL], F32, tag="t2")
                nc.vector.tensor_mul(t2[:], t1[:], q[:])
                nc.vector.tensor_reduce(s_t[:],
                                        t2[:].rearrange("p (d e) -> p d e",
                                                        e=NE),
                                        AX.X, op=OP.add)
            else:
                # final chunk: half-major e phase in two separate PSUM tiles
                # (avoids a tile-granularity WAR hazard) so the final
                # reduction chain pipelines with the PE instead of
                # serializing after it
                for half in range(2):
                    fsl = slice(half * 512, (half + 1) * 512)
                    dsl = slice(half * (DL // 2), (half + 1) * (DL // 2))
                    eph = pepool.tile([128, 512], F32, tag=f"eh{half}")
                    for j in range(4):
                        nc.tensor.matmul(eph[:], x88_t[:, :, j, :],
                                         wea8_t[:, :, j, fsl],
                                         start=(j == 0), stop=False,
                                         perf_mode=DR)
                        nc.tensor.matmul(eph[:], x88_t[:, :, j, :],
                                         web8_t[:, :, j, fsl],
                                         start=False, stop=False,
                                         perf_mode=DR)
                        nc.tensor.matmul(eph[:], xb8_t[:, :, j, :],
                                         wea8_t[:, :, j, fsl],
                                         start=False, stop=False,
                                         perf_mode=DR)
                    nc.tensor.matmul(eph[:], ones2[:],
                                     ccsb[:, half * 512:(half + 1) * 512],
                                     start=False, stop=True)
                    t1 = epool.tile([128, 512], F32, tag="t1h")
                    nc.vector.tensor_mul(t1[:], mask[:, fsl], eph[:])
                    t2 = epool.tile([128, 512], F32, tag="t2h")
                    nc.vector.tensor_mul(t2[:], t1[:], q[:, fsl])
                    nc.vector.tensor_reduce(
                        s_t[:, dsl],
                        t2[:].rearrange("p (d e) -> p d e", e=NE),
                        AX.X, op=OP.add)

            ev12 = epool.tile([128, 2 * DL], F32, tag="ev12")
            nc.scalar.activation(ev12[:, :DL], v1[:], ACTF.Exp, scale=ISC)
            nc.scalar.activation(ev12[:, DL:], v2[:], ACTF.Exp, scale=ISC)
            z_t = epool.tile([128, DL], F32, tag="z")
            nc.vector.tensor_add(z_t[:], ev12[:, :DL], ev12[:, DL:])
            r_t = epool.tile([128, DL], F32, tag="r")
            nc.vector.reciprocal(r_t[:], z_t[:])
            o_t = epool.tile([128, DL], F32, tag="o")
            # 2^-10 descales the e psum chain; folded into the mean scalar
            nc.vector.scalar_tensor_tensor(o_t[:], s_t[:],
                                           (2.0 ** -10) / NE, r_t[:],
                                           OP.mult, OP.mult)
            nc.sync.dma_start(out_c.ap()[tsl, :], o_t[:])

        # steady state: gating and e phases of each chunk back to back; the
        # last two chunks are software-pipelined (both gating stages first)
        # so the final mask chains hide under PE work instead of trailing it
        for m in range(MCH - 2):
            finish_stage(gating_stage(m), half_major=False)
        c_a = gating_stage(MCH - 2)
        c_b = gating_stage(MCH - 1)
        finish_stage(c_a, half_major=False)
        finish_stage(c_b, half_major=True)

    nc.compile()
    return nc


def _get_program():
    if "nc" not in _CACHE:
        _CACHE["nc"] = _build()
    return _CACHE["nc"]


def _prep_shared(u):
    f32 = np.float32
    E4 = ml_dtypes.float8_e4m3
    u2 = np.ascontiguousarray(np.asarray(u, dtype=f32).reshape(S, KU))
    xh16 = u2.astype(np.float16)                      # [S, K]
    xl = (u2 - xh16.astype(f32)).astype(f32)

    def dr_layout(a):
        # DoubleRow layout [p, i, j, t]: k = j*256 + i*128 + p
        return np.ascontiguousarray(
            a.reshape(4, 2, 128, S).transpose(2, 1, 0, 3))

    # main layout [p, kc, t]: k = kc*128 + p
    xhT = np.ascontiguousarray(xh16.T.reshape(8, 128, S).transpose(1, 0, 2))
    xl8T = dr_layout((xl.T * f32(2.0 ** 14)).astype(E4))
    x88 = (u2.T * f32(2.0 ** 3)).astype(E4)
    # e-pass x residual chunk (8-bit total), same 2^3 scale as x88
    xb = (u2.T - x88.astype(f32) * f32(2.0 ** -3)).astype(f32)
    xb8T = dr_layout((xb * f32(2.0 ** 3)).astype(E4))
    x88T = dr_layout(x88)
    return u2, xhT, xl8T, x88T, xb8T


def _prep_gating_w(Wu, scale):
    # Wu: [FL, KU] fp32 feature-slice of a gating projection (u-part)
    f32 = np.float32
    E4 = ml_dtypes.float8_e4m3
    WuT = np.ascontiguousarray(Wu.T.astype(f32))      # [K, F]
    wh16 = (WuT * f32(scale)).astype(np.float16)
    wA = (wh16.astype(f32) * f32(1.0 / scale)).astype(f32)
    wl = (WuT - wA).astype(f32)
    w16 = np.ascontiguousarray(wh16.reshape(8, 128, FL).transpose(1, 0, 2))
    wh8 = np.ascontiguousarray(
        (wA * f32(2.0 ** 4)).astype(E4)
        .reshape(4, 2, 128, FL).transpose(2, 1, 0, 3))
    wl8 = np.ascontiguousarray(
        (wl * f32(2.0 ** 15)).astype(E4)
        .reshape(4, 2, 128, FL).transpose(2, 1, 0, 3))
    return w16, wh8, wl8


def kernel(h, us, ue, u, noise, W_nn, b_nn, W_no, b_no, W_E, b_E, W_r, b_r):
    from concourse.bass_utils import run_bass_kernel_spmd

    f32 = np.float32
    u2, xhT, xl8T, x88T, xb8T = _prep_shared(u)

    # host R-path in fp64 (token-independent, ~4 MFLOP)
    hx = np.concatenate([np.asarray(h, dtype=f32).ravel(),
                         np.asarray(us, dtype=f32).ravel(),
                         np.asarray(ue, dtype=f32).ravel()]).astype(np.float64)
    R = hx @ np.asarray(W_r, dtype=np.float64).T + np.asarray(
        b_r, dtype=np.float64)                        # [512]

    W_nn = np.asarray(W_nn, dtype=f32)
    W_no = np.asarray(W_no, dtype=f32)
    W_E = np.asarray(W_E, dtype=f32)
    noise4 = np.asarray(noise, dtype=f32).reshape(S, DIM, NE)

    def c_of(W, b, fsl):
        return (np.asarray(b, np.float64)[fsl]
                + R @ np.asarray(W, np.float64)[fsl, KU:].T)

    def cc_rows(W, b, fsl, scale):
        c32 = (c_of(W, b, fsl) * scale).astype(f32)
        ch = _trunc11(c32)
        cl = _trunc11((c32.astype(np.float64) - ch).astype(f32))
        return ch, cl

    in_maps = []
    for c in range(NCORES):
        fsl = slice(c * FL, (c + 1) * FL)
        E4 = ml_dtypes.float8_e4m3
        wnn16, wh8nn, wl8nn = _prep_gating_w(W_nn[fsl, :KU], SC)
        wno16, wh8no, wl8no = _prep_gating_w(W_no[fsl, :KU], SC)
        # e-weights as two e4m3 chunks at scale 2^7 (psum scale 2^10)
        weT = np.ascontiguousarray(W_E[fsl, :KU].T.astype(f32))
        wea = (weT * f32(2.0 ** 7)).astype(E4)
        web = ((weT - wea.astype(f32) * f32(2.0 ** -7))
               * f32(2.0 ** 7)).astype(E4)

        def dr_w(a):
            return np.ascontiguousarray(
                a.reshape(4, 2, 128, FL).transpose(2, 1, 0, 3))

        ch_e, cl_e = cc_rows(W_E, b_E, fsl, 2.0 ** 10)
        ccd = np.stack([ch_e, cl_e]).astype(f32)
        nz_sl = noise4[:, c * DL:(c + 1) * DL, :].reshape(S, FL)
        nzc = ((c_of(W_nn, b_nn, fsl)[None, :]
                + c_of(W_no, b_no, fsl)[None, :] * nz_sl.astype(np.float64))
               * SC).astype(f32)
        im = {
            "xhT": xhT, "xl8T": xl8T, "x88T": x88T, "xb8T": xb8T,
            "whnn16": wnn16, "wh8nn": wh8nn, "wl8nn": wl8nn,
            "whno16": wno16, "wh8no": wh8no, "wl8no": wl8no,
            "wea8": dr_w(wea), "web8": dr_w(web),
            "noise_c": np.ascontiguousarray(nz_sl),
            "nzc_c": np.ascontiguousarray(nzc),
            "ccd": np.ascontiguousarray(ccd),
        }
        in_maps.append(im)

    nc = _get_program()
    res = run_bass_kernel_spmd(nc, in_maps, core_ids=list(range(NCORES)),
                               trace=TRACE)
    _CACHE["last_results"] = res
    out = np.empty((1, S, DIM), dtype=f32)
    for c in range(NCORES):
        out[0, :, c * DL:(c + 1) * DL] = res.results[c]["out_c"]
    return out
